# revision 1
# baseline (speedup 1.0000x reference)
"""Causal self-attention (B=4, T=2048, C=1024, H=16) on 8 TRN2 NeuronCores.

Sharding: data-parallel over B (4) x tensor-parallel over heads (2 halves of 8
heads). Core c handles batch c//2, heads 8*(c%2) .. 8*(c%2)+8. Each core runs
the full pipeline for its (batch, head-half): QKV projection, causal
attention, output projection against its 512 rows of w_proj, producing a
partial [C, T] output; the host sums core pairs and transposes.

Attention is computed in the S^T = K^T-major layout so no on-chip transposes
are needed: softmax denominators come from a ones-column appended to V, and
the division is broadcast across partitions with a rank-1 PE matmul.

All matmuls run in fp16 (fp32 PSUM accumulation). fp16 keeps 10 mantissa
bits; every tensor in this problem is O(1)-ranged so there is no overflow
risk, and measured end-to-end error vs the fp32 reference is ~1e-3.
"""

import sys

if "/opt/trn_rl_repo" not in sys.path:
    sys.path.insert(0, "/opt/trn_rl_repo")

from contextlib import ExitStack

import numpy as np

import concourse.tile as tile
from concourse import bacc, mybir

F32 = mybir.dt.float32
F32R = mybir.dt.float32r
FP16 = mybir.dt.float16

B, T, C, H = 4, 2048, 1024, 16
HL = 8  # heads per core
HD = 64  # head dim
CL = HL * HD  # local channel width (512)
NT = T // 512  # 4 t-chunks of 512
NK = C // 128  # 8 contraction tiles over C
NTT = T // 128  # 16 t1/t2 tiles of 128


def build_nc(split_moving=False):
    nc = bacc.Bacc(None)

    xT_d = nc.declare_dram_parameter("xT", [C, T], FP16, isOutput=False)
    wqk_d = nc.declare_dram_parameter("wqk", [C, 2 * CL], FP16, isOutput=False)
    wv_d = nc.declare_dram_parameter("wv", [C, CL], FP16, isOutput=False)
    wproj_d = nc.declare_dram_parameter("wproj", [HL, HD, C], FP16, isOutput=False)
    bqk_d = nc.declare_dram_parameter("bqk", [128, 8], F32, isOutput=False)
    bv_d = nc.declare_dram_parameter("bv", [128, CL], F32, isOutput=False)
    bproj_d = nc.declare_dram_parameter("bproj", [128, 8], F32, isOutput=False)
    outT_d = nc.declare_dram_parameter("outT", [C, T], F32, isOutput=True)

    with tile.TileContext(nc) as tc, ExitStack() as ctx:
        # ---------- persistent pools ----------
        persist = ctx.enter_context(tc.tile_pool(name="persist", bufs=1))
        qkT = []  # 8 tiles [128, T] fp16: rows = qkv-column block nn
        for nn in range(8):
            t_qk = persist.tile([128, T], FP16, tag=f"qkT{nn}")
            qkT.append(t_qk)
        vaug = []  # 16 tiles [128, 8*65] fp16: v (64 cols) + ones col per head
        for j in range(NTT):
            t_va = persist.tile([128, HL * 65], FP16, tag=f"vaug{j}")
            vaug.append(t_va)
        bqk_sb = persist.tile([128, 8], F32, tag="bqk")
        bv_sb = persist.tile([128, CL], F32, tag="bv")
        bproj_sb = persist.tile([128, 8], F32, tag="bproj")
        ones_f32 = persist.tile([128, HD], F32, tag="ones_f32")
        ones_sb = persist.tile([128, HD], F32R, tag="ones")
        wproj_sb = []  # per head [64, C]
        for h in range(HL):
            t_wp = persist.tile([64, C], FP16, tag=f"wproj{h}")
            wproj_sb.append(t_wp)

        nc.sync.dma_start(bqk_sb[:], bqk_d[:])
        nc.sync.dma_start(bv_sb[:], bv_d[:])
        nc.sync.dma_start(bproj_sb[:], bproj_d[:])
        nc.vector.memset(ones_f32[:], 1.0)
        nc.vector.tensor_copy(ones_sb[:], ones_f32[:])
        for h in range(HL):
            nc.sync.dma_start(wproj_sb[h][:], wproj_d[h])
        for j in range(NTT):
            # ones columns (64::65) for the PV row-sum trick
            nc.vector.memset(vaug[j][:, 64 :: 65], 1.0)

        # y storage: per head [64, T] fp16 (base partition 0 so DVE ops stay
        # partition-aligned with the [65, 512] PSUM accumulators)
        yT_sb = []
        for h in range(HL):
            t_y = persist.tile([64, T], FP16, tag=f"yT{h}")
            yT_sb.append(t_y)

        with (
            tc.tile_pool(name="wqks", bufs=1) as wqks,
            tc.tile_pool(name="xts", bufs=1) as xts,
            tc.tile_pool(name="pts", bufs=6) as pts,
            tc.tile_pool(name="rts", bufs=3) as rts,
            tc.tile_pool(name="ots", bufs=3) as ots,
            tc.tile_pool(name="pp", bufs=2, space="PSUM") as pp,
            tc.tile_pool(name="ps_y", bufs=2, space="PSUM") as ps_y,
            tc.tile_pool(name="ps_b", bufs=1, space="PSUM") as ps_b,
            tc.tile_pool(name="ps_o", bufs=1, space="PSUM") as ps_o,
        ):
            # resident x^T (ordered so chunk 0 lands first) and weights
            wv_sb = []
            for kc in range(NK):
                t_wv = wqks.tile([128, CL], FP16, tag=f"wv{kc}")
                nc.sync.dma_start(t_wv[:], wv_d[kc * 128 : (kc + 1) * 128, :])
                wv_sb.append(t_wv)
            xt = [[None] * NK for _ in range(NT)]
            for tc_ in range(NT):
                for kc in range(NK):
                    t_x = xts.tile([128, 512], FP16, tag=f"xt{tc_}_{kc}")
                    nc.sync.dma_start(
                        t_x[:],
                        xT_d[kc * 128 : (kc + 1) * 128, tc_ * 512 : (tc_ + 1) * 512],
                    )
                    xt[tc_][kc] = t_x
            wqk_sb = []
            for kc in range(NK):
                t_wqk = wqks.tile([128, 2 * CL], FP16, tag=f"wqk{kc}")
                nc.sync.dma_start(t_wqk[:], wqk_d[kc * 128 : (kc + 1) * 128, :])
                wqk_sb.append(t_wqk)

            # ---- v first: unlocks PV for every head ----
            for tc_ in range(NT):
                for ti in range(4):
                    j = tc_ * 4 + ti
                    p_v = pp.tile([128, 1024], F32, tag="pp")
                    for kc in range(NK):
                        nc.tensor.matmul(
                            p_v[:, 0:512],
                            xt[tc_][kc][:, ti * 128 : (ti + 1) * 128],
                            wv_sb[kc][:],
                            start=(kc == 0),
                            stop=(kc == NK - 1),
                        )
                    nc.vector.tensor_tensor(
                        vaug[j][:].rearrange("p (h c) -> p h c", h=HL)[:, :, 0:64],
                        p_v[:, 0:512].rearrange("p (h c) -> p h c", h=HL),
                        bv_sb[:].rearrange("p (h c) -> p h c", h=HL),
                        mybir.AluOpType.add,
                    )

            def qk_block(nn):
                # q^T (or k^T) block nn over all t, paired t-chunks per psum
                for tcp in range(2):
                    p_qk = pp.tile([128, 1024], F32, tag="pp")
                    for half in range(2):
                        tc_ = 2 * tcp + half
                        for kc in range(NK):
                            nc.tensor.matmul(
                                p_qk[:, half * 512 : half * 512 + 512],
                                wqk_sb[kc][:, nn * 128 : (nn + 1) * 128],
                                xt[tc_][kc][:],
                                start=(kc == 0),
                                stop=(kc == NK - 1),
                            )
                    nc.vector.tensor_scalar_add(
                        qkT[nn][:, tcp * 1024 : (tcp + 1) * 1024],
                        p_qk[:],
                        bqk_sb[:, nn : nn + 1],
                    )

            def attention(h):
                poff = (h % 2) * 64
                kt = qkT[4 + h // 2]
                qt = qkT[h // 2]
                for c in range(NT):
                    p_y = ps_y.tile([65, 512], F32, tag="py")
                    nj = 4 * c + 4
                    for j0 in range(0, nj, 2):
                        p_s = pp.tile([128, 1024], F32, tag="pp")
                        for half in range(2):
                            j = j0 + half
                            nc.tensor.matmul(
                                p_s[:, half * 512 : half * 512 + 512],
                                kt[poff : poff + 64, j * 128 : (j + 1) * 128],
                                qt[poff : poff + 64, c * 512 : (c + 1) * 512],
                                start=True,
                                stop=True,
                            )
                        pt = pts.tile([128, 1024], FP16, tag="pt")
                        nc.scalar.activation(
                            pt[:], p_s[:], mybir.ActivationFunctionType.Exp
                        )
                        if j0 // 4 == c:
                            # zero strictly-upper entries of both halves:
                            # keep where 512c - 128(j0+half) + f - p >= 0
                            ptm = pts.tile([128, 1024], FP16, tag="ptm")
                            nc.gpsimd.affine_select(
                                ptm[:].rearrange("p (s f) -> p s f", s=2),
                                pt[:].rearrange("p (s f) -> p s f", s=2),
                                pattern=[[-128, 2], [1, 512]],
                                compare_op=mybir.AluOpType.is_ge,
                                fill=0.0,
                                base=512 * c - 128 * j0,
                                channel_multiplier=-1,
                            )
                            pt = ptm
                        for half in range(2):
                            j = j0 + half
                            nc.tensor.matmul(
                                p_y[:],
                                vaug[j][:, h * 65 : (h + 1) * 65],
                                pt[:, half * 512 : half * 512 + 512],
                                start=(j == 0),
                                stop=(j == nj - 1),
                            )
                    # normalize: y[d, t] = y_aug[d, t] / y_aug[64, t]
                    r_sb = rts.tile([128, 512], F32R, tag="r")
                    with nc.allow_low_precision(
                        reason="f32r holds full fp32-rounded reciprocal"
                    ):
                        nc.vector.reciprocal(r_sb[64:65, :], p_y[64:65, :])
                    p_r = ps_b.tile([64, 512], F32, tag="pb")
                    nc.tensor.matmul(
                        p_r[:],
                        ones_sb[64:65, :],
                        r_sb[64:65, :],
                        start=True,
                        stop=True,
                    )
                    rb_sb = rts.tile([64, 512], F32, tag="rb")
                    nc.vector.tensor_copy(rb_sb[:], p_r[:])
                    nc.vector.tensor_mul(
                        yT_sb[h][:, c * 512 : (c + 1) * 512], p_y[0:64, :], rb_sb[:]
                    )

            # interleave: each qk n-tile pair unlocks two heads of attention
            for i in range(4):
                qk_block(i)
                qk_block(4 + i)
                attention(2 * i)
                attention(2 * i + 1)

            # ---- proj ----
            for c in range(NT):
                for co in range(8):
                    p_o = ps_o.tile([128, 512], F32, tag="po")
                    for h in range(HL):
                        nc.tensor.matmul(
                            p_o[:],
                            wproj_sb[h][:, co * 128 : (co + 1) * 128],
                            yT_sb[h][:, c * 512 : (c + 1) * 512],
                            start=(h == 0),
                            stop=(h == HL - 1),
                        )
                    o_sb = ots.tile([128, 512], F32, tag="o")
                    nc.vector.tensor_scalar_add(
                        o_sb[:], p_o[:], bproj_sb[:, co : co + 1]
                    )
                    nc.sync.dma_start(
                        outT_d[co * 128 : (co + 1) * 128, c * 512 : (c + 1) * 512],
                        o_sb[:],
                    )

    nc.compile()
    return nc


# ---------------------------------------------------------------------------
# host side
# ---------------------------------------------------------------------------

_CACHE = {}


def _get_runner():
    if "runner" in _CACHE:
        return _CACHE["runner"]

    import jax
    from jax.experimental.shard_map import shard_map
    from jax.sharding import Mesh, PartitionSpec

    from concourse.bass2jax import (
        _bass_exec_p,
        install_neuronx_cc_hook,
        partition_id_tensor,
    )

    install_neuronx_cc_hook()
    nc = build_nc()
    n_cores = 8

    partition_name = nc.partition_id_tensor.name if nc.partition_id_tensor else None
    in_names = []
    out_names = []
    out_avals = []
    for alloc in nc.m.functions[0].allocations:
        if not isinstance(alloc, mybir.MemoryLocationSet):
            continue
        name = alloc.memorylocations[0].name
        if alloc.kind == "ExternalInput":
            if name != partition_name:
                in_names.append(name)
        elif alloc.kind == "ExternalOutput":
            out_names.append(name)
            out_avals.append(
                jax.core.ShapedArray(tuple(alloc.tensor_shape), mybir.dt.np(alloc.dtype))
            )
    n_params = len(in_names)
    all_names = in_names + out_names
    if partition_name is not None:
        all_names = all_names + [partition_name]

    def _body(*args):
        operands = list(args)
        if partition_name is not None:
            operands.append(partition_id_tensor())
        outs = _bass_exec_p.bind(
            *operands,
            out_avals=tuple(out_avals),
            in_names=tuple(all_names),
            out_names=tuple(out_names),
            lowering_input_output_aliases=(),
            sim_require_finite=True,
            sim_require_nnan=True,
            nc=nc,
        )
        return tuple(outs)

    devices = jax.devices()[:n_cores]
    mesh = Mesh(np.asarray(devices), ("core",))
    n_outs = len(out_names)
    fn = jax.jit(
        shard_map(
            _body,
            mesh=mesh,
            in_specs=(PartitionSpec("core"),) * (n_params + n_outs),
            out_specs=(PartitionSpec("core"),) * n_outs,
            check_rep=False,
        ),
        keep_unused=True,
    )

    runner = {
        "fn": fn,
        "in_names": in_names,
        "out_names": out_names,
        "out_avals": out_avals,
        "n_cores": n_cores,
        "jax": jax,
    }
    _CACHE["runner"] = runner
    return runner


def _prepare_in_maps(x, w_attn, b_attn, w_proj, b_proj):
    x = np.asarray(x, dtype=np.float32)
    w_attn = np.asarray(w_attn, dtype=np.float32)
    b_attn = np.asarray(b_attn, dtype=np.float32)
    w_proj = np.asarray(w_proj, dtype=np.float32)
    b_proj = np.asarray(b_proj, dtype=np.float32)

    in_maps = []
    for core in range(8):
        b = core // 2
        h0 = HL * (core % 2)
        c0 = h0 * HD  # 512*(core%2)

        xT = np.ascontiguousarray(x[b].T).astype(np.float16)

        w_q = (w_attn[:, c0 : c0 + CL] * 0.125).astype(np.float16)
        w_k = w_attn[:, C + c0 : C + c0 + CL].astype(np.float16)
        wqk = np.ascontiguousarray(np.concatenate([w_q, w_k], axis=1))
        wv = np.ascontiguousarray(w_attn[:, 2 * C + c0 : 2 * C + c0 + CL]).astype(
            np.float16
        )
        wproj = np.ascontiguousarray(
            w_proj[c0 : c0 + CL, :].reshape(HL, HD, C)
        ).astype(np.float16)

        b_q = b_attn[c0 : c0 + CL] * 0.125
        b_k = b_attn[C + c0 : C + c0 + CL]
        bqk = np.concatenate([b_q, b_k]).reshape(8, 128).T.astype(np.float32)
        bqk = np.ascontiguousarray(bqk)
        b_v = b_attn[2 * C + c0 : 2 * C + c0 + CL].astype(np.float32)
        bv = np.ascontiguousarray(np.broadcast_to(b_v[None, :], (128, CL)))
        if core % 2 == 0:
            bp = np.ascontiguousarray(b_proj.reshape(8, 128).T.astype(np.float32))
        else:
            bp = np.zeros((128, 8), dtype=np.float32)

        in_maps.append(
            {
                "xT": xT,
                "wqk": wqk,
                "wv": wv,
                "wproj": wproj,
                "bqk": bqk,
                "bv": bv,
                "bproj": bp,
            }
        )
    return in_maps


def _run_device(in_maps):
    r = _get_runner()
    jax = r["jax"]
    n = r["n_cores"]
    per_core = [[np.asarray(m[name]) for name in r["in_names"]] for m in in_maps]
    concat_in = [
        np.concatenate([per_core[c][i] for c in range(n)], axis=0)
        for i in range(len(r["in_names"]))
    ]
    concat_zero = [
        np.zeros((n * a.shape[0], *a.shape[1:]), a.dtype) for a in r["out_avals"]
    ]
    outs = r["fn"](*[jax.device_put(a) for a in concat_in + concat_zero])
    jax.block_until_ready(outs)
    (outT,) = [np.asarray(o) for o in outs]
    return outT.reshape(n, C, T)


def kernel(x, w_attn, b_attn, w_proj, b_proj):
    in_maps = _prepare_in_maps(x, w_attn, b_attn, w_proj, b_proj)
    outT = _run_device(in_maps)
    # host gather: sum the two head-halves of each batch, transpose back
    out = np.empty((B, T, C), dtype=np.float32)
    for b in range(B):
        out[b] = (outT[2 * b] + outT[2 * b + 1]).T
    return out

